# revision 24
# baseline (speedup 1.0000x reference)
"""ECE loss kernel for Trainium2, data-parallel over 8 NeuronCores.

Host side shards + permutes samples (binning is permutation invariant) into
128-sample "slots", swaps each row's label class into column 0 (max/sum are
class-permutation invariant, so accuracy becomes one column compare), casts
logits to bf16, and stores each 16-slot block's classes split into two
50-class halves so one fp8 DoubleRow matmul pair-sums them.

Device per tile (128 partitions x 128 slots x 100 classes):
  ScalarE: one exp(logits/T) instruction per tile -> E in fp8e4m3, plus a
           tiny per-pair exp of the raw per-sample max.
  DVE:     pairwise max tree on the RAW bf16 logits (independent of exp),
           fast reciprocal, conf/acc, 15 bin-threshold masks.
  PE:      softmax denominators via fp8 DoubleRow identity matmuls that
           pair-sum the two class halves while accumulating into PSUM with a
           stride-0 output dim (ping-pong over 2 partials: distance-2 RMW),
           plus the per-(slot, bin) histogram matmuls into 4 PSUM banks.
Pad rows are col0=0, rest=-80: conf=1.0 and acc=1 exactly, cancelling in the
bin-15 differential.  Each core DMAs its [4,128,480] histogram out; the host
extracts diagonal slot blocks and finishes the ECE reduction in float64.
"""

import hashlib
import sys

import numpy as np

sys.path.insert(0, "/opt/trn_rl_repo")

import ml_dtypes  # noqa: E402

from concourse import bacc, bass, mybir, tile  # noqa: E402
from concourse import bass_utils  # noqa: E402

P = 128          # partitions
G = 128          # slots (groups) per tile
TILE = P * G     # samples per tile (16384)
C = 100          # classes
H = C // 2       # class half (50)
BS = 16          # slots per block (DMA/exp/L1 pipeline granularity)
NBLK = G // BS   # blocks per tile
NBINS = 15
N_CORES = 8
BIG = 80.0       # pad logit; exp(-80) flushes to 0 in fp8/bf16 sums
N_TOTAL = 2_000_000
MM_SLOTS = 10    # slots per S-matmul (out iterations 10*50 <= 512)
G_LAST = 48      # short final tile (slots); 15*G + G_LAST = 1968 slots/core
NBLK_LAST = G_LAST // BS
N_TILES = 16
SLOTS_CORE = 15 * G + G_LAST

F32 = mybir.dt.float32
BF16 = mybir.dt.bfloat16
FP8 = mybir.dt.float8e4
BFNP = ml_dtypes.bfloat16
F8NP = ml_dtypes.float8_e4m3fn
AX = mybir.AxisListType
ALU = mybir.AluOpType
ACTF = mybir.ActivationFunctionType
PM = mybir.MatmulPerfMode


# ---------------------------------------------------------------- host layout

def build_plan(labels: np.ndarray, n_cores: int = N_CORES):
    """Deal samples round-robin to cores; slots mix labels freely (the
    label-column swap is per-row).  Returns per-core padded index arrays."""
    labels = np.asarray(labels).astype(np.int64).ravel()
    N = labels.shape[0]
    cap = SLOTS_CORE * P
    core_idx = []
    for c in range(n_cores):
        idx = np.arange(c, N, n_cores, dtype=np.int64)
        assert len(idx) <= cap
        buf = np.full(cap, -1, dtype=np.int64)
        buf[:len(idx)] = idx
        core_idx.append(buf)
    return core_idx


def build_core_slab(logits: np.ndarray, labels: np.ndarray,
                    idx: np.ndarray) -> np.ndarray:
    """One core's bf16 slab in device DMA order: 15 full tiles then one
    G_LAST tile, each [P, nblk, 2, BS, H], label class swapped into col 0."""
    cap = len(idx)
    arr = np.empty((cap, C), dtype=BFNP)
    arr[:, :] = logits[np.maximum(idx, 0)].astype(BFNP)
    ks = np.asarray(labels).astype(np.int64)[np.maximum(idx, 0)]
    rows = np.arange(cap)
    col0 = arr[rows, 0].copy()
    arr[rows, 0] = arr[rows, ks]
    arr[rows, ks] = col0
    pad_pos = np.nonzero(idx < 0)[0]
    if len(pad_pos):
        # col0=0, rest=-BIG: conf=1.0 and acc=1 exactly (cancels in bin 15)
        arr[pad_pos, :C] = BFNP(-BIG)
        arr[pad_pos, 0] = BFNP(0.0)
    full = arr[:15 * G * P].reshape(15, NBLK, BS, P, 2, H)
    full = np.ascontiguousarray(full.transpose(0, 3, 1, 4, 2, 5))
    last = arr[15 * G * P:].reshape(1, NBLK_LAST, BS, P, 2, H)
    last = np.ascontiguousarray(last.transpose(0, 3, 1, 4, 2, 5))
    return np.concatenate([full.reshape(-1, H), last.reshape(-1, H)])


# ------------------------------------------------------------- device program

def build_program(T: int, n_cores: int = N_CORES):
    nc = bacc.Bacc("TRN2", target_bir_lowering=False, debug=False,
                   num_devices=n_cores)

    logits_d = nc.dram_tensor("logits", [SLOTS_CORE * P * 2, H], BF16,
                              kind="ExternalInput")
    tempr_d = nc.dram_tensor("tempr", [P, 1], F32, kind="ExternalInput")
    eye8_d = nc.dram_tensor("eye8", [P, 2 * 128], FP8, kind="ExternalInput")
    out_d = nc.dram_tensor("out", [4, P, 32 * NBINS], F32,
                           kind="ExternalOutput")

    thr_imm = [float(np.float32(BFNP(b / NBINS))) for b in range(NBINS)]
    assert T % 2 == 0
    TF = G * C          # free elems per tile (12800)
    BF = BS * C         # free elems per block (1600)
    tile_G = [G] * (T - 1) + [G_LAST]
    tile_nblk = [NBLK] * (T - 1) + [NBLK_LAST]
    row_off = [0]
    for t in range(T):
        row_off.append(row_off[-1] + P * tile_nblk[t] * 2 * BS)

    with tile.TileContext(nc) as tc:
        with (
            tc.tile_pool(name="const", bufs=1) as const,
            tc.tile_pool(name="rawp", bufs=3) as rawp,
            tc.tile_pool(name="sb", bufs=2) as sbp,
            tc.tile_pool(name="ps", bufs=1, space="PSUM") as psp,
        ):
            zw = const.tile([P, 1], F32)
            nc.vector.memset(zw, 0.0)
            warm = const.tile([P, 1], F32)
            nc.scalar.activation(warm, zw, ACTF.Exp)
            tempr_t = const.tile([P, 1], F32)
            nc.sync.dma_start(tempr_t, tempr_d.ap())
            invT = const.tile([P, 1], F32)
            nc.vector.reciprocal(invT, tempr_t)
            eye8 = const.tile([P, 2 * 128], FP8)
            nc.sync.dma_start(eye8, eye8_d.ap())
            lhsT8 = eye8.rearrange("p (kt m) -> p kt m", kt=2)
            zeros_w = const.tile([P, 128], BF16)
            nc.vector.memset(zeros_w, 0.0)
            zdummy = const.tile([P, 512], BF16)
            nc.vector.memset(zdummy, 0.0)

            hists = [psp.tile([P, 32 * NBINS], F32, name=f"hist{q}")
                     for q in range(4)]  # [mainA, mainB, spareA, spareB]

            logits_ap = logits_d.ap()

            def tail_h(tp, h, Gt, Sp4, pack, emax_h):
                t = 2 * tp + h
                Sc = sbp.tile([P, G], F32, tag="Sch", name="Sch", bufs=2)
                nc.vector.tensor_copy(Sc[:, 0:Gt], Sp4[:, h, 1, 0:Gt])
                S2 = sbp.tile([P, G], F32, tag="S2h", name="S2h", bufs=2)
                nc.vector.tensor_tensor(S2[:, 0:Gt], Sp4[:, h, 0, 0:Gt],
                                        Sc[:, 0:Gt], op=ALU.add)
                R2 = sbp.tile([P, G], F32, tag="R2h", name="R2h", bufs=2)
                nc.vector.reciprocal_approx_fast(R2[:, 0:Gt], S2[:, 0:Gt])
                Rb2 = sbp.tile([P, G], BF16, tag="Rb2h", name="Rb2h", bufs=2)
                nc.vector.tensor_copy(Rb2[:, 0:Gt], R2[:, 0:Gt])
                conf = sbp.tile([P, G], BF16, tag="confh", name="confh",
                                bufs=2)
                nc.vector.tensor_tensor(conf[:, 0:Gt], emax_h[:, 0:Gt],
                                        Rb2[:, 0:Gt], op=ALU.mult)
                maskh = sbp.tile([P, NBINS * G], BF16, tag="maskh",
                                 name="maskh", bufs=2)
                for b in range(NBINS):
                    nc.vector.tensor_scalar(
                        maskh[:, b * G:b * G + Gt], conf[:, 0:Gt],
                        thr_imm[b], None, op0=ALU.is_gt)
                mh3 = maskh.rearrange("p (b g) -> p b g", b=NBINS)
                pack4 = pack.rearrange("p (r g) -> p r g", r=4)
                if Gt == G:
                    nc.vector.tensor_copy(
                        pack4[:, 0:3:2, :],
                        conf.rearrange("p (u g) -> p u g", u=2))
                else:
                    nc.vector.tensor_copy(pack[:, 0:Gt], conf[:, 0:Gt])
                for q in range(4):
                    if Gt < G and q >= 2:
                        continue
                    u = q // 2
                    w = min(32, max(0, Gt - 32 * q))
                    lhsT = pack[:, 128 * u:128 * u + 128]
                    rhs = mh3[:, :, 32 * q:32 * q + w]
                    ob = (hists[q] if w == 32 else
                          hists[q].rearrange("p (b j) -> p b j",
                                             b=NBINS)[:, :, 0:w])
                    stop_t = T - 2 if q >= 2 else T - 1
                    nc.tensor.matmul(ob, lhsT=lhsT, rhs=rhs,
                                     start=(t == 0), stop=(t == stop_t))
                if t == T - 2:
                    # banks 2,3 are complete: drain them now
                    for q in (2, 3):
                        hsb = sbp.tile([P, 32 * NBINS], F32, tag="hsb",
                                       name="hsb", bufs=4)
                        nc.vector.tensor_copy(hsb, hists[q])
                        nc.sync.dma_start(out_d.ap()[q], hsb)

            def tail(ctx):
                tp, m2, Sp4, pack_h, emax2 = ctx
                Sc = sbp.tile([P, 2 * G], F32, tag="Sc", name="Sc", bufs=2)
                Sc3 = Sc.rearrange("p (h g) -> p h g", h=2)
                nc.vector.tensor_copy(Sc3, Sp4[:, :, 1, :])
                S2 = sbp.tile([P, 2 * G], F32, tag="S2", name="S2", bufs=2)
                S23 = S2.rearrange("p (h g) -> p h g", h=2)
                nc.vector.tensor_tensor(S23, Sp4[:, :, 0, :], Sc3,
                                        op=ALU.add)
                R2 = sbp.tile([P, 2 * G], F32, tag="R2", name="R2", bufs=2)
                nc.vector.reciprocal_approx_fast(R2, S2)
                Rb2 = sbp.tile([P, 2 * G], BF16, tag="Rb2", name="Rb2",
                               bufs=2)
                nc.vector.tensor_copy(Rb2, R2)
                conf2 = sbp.tile([P, 2 * G], BF16, tag="conf2", name="conf2",
                                 bufs=2)
                nc.vector.tensor_tensor(conf2, emax2, Rb2, op=ALU.mult)

                # cumulative bin masks: 15 tensor_scalar is_gt (4x mode)
                mask2 = sbp.tile([P, NBINS * 2 * G], BF16, tag="mask2",
                                 name="mask2")
                for b in range(NBINS):
                    nc.vector.tensor_scalar(
                        mask2[:, b * 2 * G:(b + 1) * 2 * G], conf2,
                        thr_imm[b], None, op0=ALU.is_gt)
                m3 = mask2.rearrange("p (b g) -> p b g", b=NBINS)

                for h in range(2):
                    t = 2 * tp + h
                    Gt = tile_G[t]
                    pack = pack_h[h]
                    pack4 = pack.rearrange("p (r g) -> p r g", r=4)
                    if Gt == G:
                        nc.vector.tensor_copy(
                            pack4[:, 0:3:2, :],
                            conf2[:, h * G:(h + 1) * G].rearrange(
                                "p (u g) -> p u g", u=2))
                    else:
                        nc.vector.tensor_copy(pack[:, 0:Gt],
                                              conf2[:, h * G:h * G + Gt])

                    # histogram matmuls: slot-quarters into 4 PSUM banks,
                    # accumulated across all tiles.  Banks 2,3 get their last
                    # contribution from the T-2 tile (the short tile only
                    # fills banks 0,1).
                    for q in range(4):
                        if Gt < G and q >= 2:
                            continue
                        u = q // 2
                        w = min(32, max(0, Gt - 32 * q))
                        lhsT = pack[:, 128 * u:128 * u + 128]
                        rhs = m3[:, :, h * G + 32 * q:h * G + 32 * q + w]
                        ob = (hists[q] if w == 32 else
                              hists[q].rearrange("p (b j) -> p b j",
                                                 b=NBINS)[:, :, 0:w])
                        stop_t = T - 2 if q >= 2 else T - 1
                        nc.tensor.matmul(ob, lhsT=lhsT, rhs=rhs,
                                         start=(t == 0), stop=(t == stop_t))

            pending = None
            for tp in range(T // 2):
                m2 = sbp.tile([P, 2 * G], BF16, tag="m2", name="m2", bufs=2)
                # S partials for the pair: [h][two][g] f32, one PSUM bank
                Spair = psp.tile([P, 2 * 2 * G], F32, tag="Spair",
                                 name="Spair", bufs=2)
                # zero the bank on PE itself (keeps the S pipeline off DVE)
                nc.tensor.matmul(Spair, lhsT=zeros_w, rhs=zdummy,
                                 start=True, stop=False,
                                 skip_group_check=True)
                Sp4 = Spair.rearrange("p (h two g) -> p h two g", h=2, two=2)
                pack_h = []
                raw0_h = []
                emax_hh = []
                for h in range(2):
                    t = 2 * tp + h
                    Gt = tile_G[t]
                    nbt = tile_nblk[t]
                    raw = rawp.tile([P, Gt * C], BF16, tag="raw", name="raw",
                                    padded_shape=[P, TF])
                    raw5 = raw.rearrange("p (blk half s c) -> p blk half s c",
                                         blk=nbt, half=2, s=BS)
                    raw0_h.append(raw5[:, :, 0, :, 0])
                    E = sbp.tile([P, Gt * C], FP8, tag="E", name="E", bufs=3,
                                 padded_shape=[P, TF])
                    E3 = E.rearrange("p (blk half n) -> p blk half n",
                                     blk=nbt, half=2)
                    t1 = sbp.tile([P, Gt * H], BF16, tag="t1", name="t1",
                                  bufs=1, padded_shape=[P, G * H])
                    t13 = t1.rearrange("p (g c) -> p g c", g=Gt)
                    t14 = t1.rearrange("p (blk s c) -> p blk s c",
                                       blk=nbt, s=BS)

                    # Early tiles are sub-chunked so DVE/PE start earlier.
                    src = logits_ap[row_off[t]:row_off[t + 1],
                                    :].rearrange("(p f) c -> p (f c)", p=P)
                    nblk_per = {0: 1, 1: 2, 2: 4, T - 1: 1}.get(t, nbt)
                    for b0 in range(0, nbt, nblk_per):
                        b1 = b0 + nblk_per
                        fsl = slice(b0 * BF, b1 * BF)
                        nc.sync.dma_start(raw[:, fsl], src[:, fsl])
                        nc.scalar.activation(E[:, fsl], raw[:, fsl], ACTF.Exp,
                                             scale=invT)
                        # max tree L1 on raw halves (bf16 2x TT)
                        nc.vector.tensor_tensor(
                            t14[:, b0:b1], raw5[:, b0:b1, 0],
                            raw5[:, b0:b1, 1], op=ALU.max)

                    # ---- per-sample sum on PE: fp8 DoubleRow pair-summing
                    # matmuls, ping-pong partials (distance-2 RMW)
                    Sh = Sp4[:, h].transpose([0, 2, 1])  # [p, g, two]
                    for blk in range(nbt):
                        for s0 in range(0, BS, MM_SLOTS):
                            s1 = min(s0 + MM_SLOTS, BS)
                            g0 = blk * BS + s0
                            g1 = blk * BS + s1
                            rhs = E3[:, blk, :, s0 * H:s1 * H]
                            dst = (Sh[:, g0:g1, :].unsqueeze(2)
                                   .broadcast_to((P, g1 - g0, H // 2, 2)))
                            nc.tensor.matmul(dst, lhsT=lhsT8, rhs=rhs,
                                             start=False, stop=True,
                                             perf_mode=PM.DoubleRow,
                                             skip_group_check=True)

                    # ---- max tree L2..L7 (overlapped splits keep alignment)
                    t2 = sbp.tile([P, Gt * 26], BF16, tag="t2", name="t2",
                                  bufs=1, padded_shape=[P, G * 26])
                    t23 = t2.rearrange("p (g c) -> p g c", g=Gt)
                    nc.vector.tensor_tensor(t23, t13[:, :, 0:26],
                                            t13[:, :, 24:50], op=ALU.max)
                    t3 = sbp.tile([P, Gt * 14], BF16, tag="t3", name="t3",
                                  bufs=1, padded_shape=[P, G * 14])
                    t33 = t3.rearrange("p (g c) -> p g c", g=Gt)
                    nc.vector.tensor_tensor(t33, t23[:, :, 0:14],
                                            t23[:, :, 12:26], op=ALU.max)
                    t4 = sbp.tile([P, Gt * 8], BF16, tag="t4", name="t4",
                                  bufs=1, padded_shape=[P, G * 8])
                    t43 = t4.rearrange("p (g c) -> p g c", g=Gt)
                    nc.vector.tensor_tensor(t43, t33[:, :, 0:8],
                                            t33[:, :, 6:14], op=ALU.max)
                    t5 = sbp.tile([P, Gt * 4], BF16, tag="t5", name="t5",
                                  bufs=1, padded_shape=[P, G * 4])
                    t53 = t5.rearrange("p (g c) -> p g c", g=Gt)
                    nc.vector.tensor_tensor(t53, t43[:, :, 0:4],
                                            t43[:, :, 4:8], op=ALU.max)
                    t6 = sbp.tile([P, Gt * 2], BF16, tag="t6", name="t6",
                                  bufs=1, padded_shape=[P, G * 2])
                    t63 = t6.rearrange("p (g c) -> p g c", g=Gt)
                    nc.vector.tensor_tensor(t63, t53[:, :, 0:2],
                                            t53[:, :, 2:4], op=ALU.max)
                    nc.vector.tensor_tensor(m2[:, h * G:h * G + Gt],
                                            t63[:, :, 0:1].opt(),
                                            t63[:, :, 1:2].opt(), op=ALU.max)

                    # acc = raw[label] >= max(raw) (label class is column 0)
                    pack = sbp.tile([P, 2 * G], BF16, tag="pack", name="pack",
                                    bufs=4)
                    pack_h.append(pack)
                    pack4 = pack.rearrange("p (r g) -> p r g", r=4)
                    if Gt == G:
                        nc.vector.tensor_tensor(
                            pack4[:, 1:4:2, :].rearrange(
                                "p r (blk s) -> p r blk s", blk=NBLK // 2),
                            raw0_h[h].rearrange("p (u blk) s -> p u blk s",
                                                u=2),
                            m2[:, h * G:(h + 1) * G].rearrange(
                                "p (u blk s) -> p u blk s", u=2,
                                blk=NBLK // 2),
                            op=ALU.is_ge)
                        if tp == T // 2 - 1:
                            emax_h = sbp.tile([P, G], BF16, tag="emaxh",
                                              name="emax_h", bufs=2)
                            nc.scalar.activation(
                                emax_h[:, 0:Gt], m2[:, h * G:h * G + Gt],
                                ACTF.Exp, scale=invT)
                            emax_hh.append(emax_h)
                    else:
                        # short tile: all slots live in the u=0 half; zero
                        # the pack so dead lhsT columns contribute nothing
                        nc.vector.memset(pack, 0.0)
                        nc.vector.tensor_tensor(
                            pack[:, 64:64 + Gt].rearrange(
                                "p (blk s) -> p blk s", blk=nbt),
                            raw0_h[h],
                            m2[:, h * G:h * G + Gt].rearrange(
                                "p (blk s) -> p blk s", blk=nbt),
                            op=ALU.is_ge)
                        if tp == T // 2 - 1:
                            emax_h = sbp.tile([P, G], BF16, tag="emaxh",
                                              name="emax_h", bufs=2)
                            nc.scalar.activation(
                                emax_h[:, 0:Gt], m2[:, h * G:h * G + Gt],
                                ACTF.Exp, scale=invT)
                            emax_hh.append(emax_h)

                if tp < T // 2 - 1:
                    emax2 = sbp.tile([P, 2 * G], BF16, tag="emax2",
                                     name="emax2", bufs=2)
                    nc.scalar.activation(emax2, m2, ACTF.Exp, scale=invT)
                    if pending is not None:
                        tail(pending)
                    pending = (tp, m2, Sp4, pack_h, emax2)
                else:
                    # last pair: per-h emax already emitted in the h-loop
                    if pending is not None:
                        tail(pending)
                    for h in range(2):
                        tail_h(tp, h, tile_G[2 * tp + h], Sp4, pack_h[h],
                               emax_hh[h])

            # ---- finalize: drain the remaining histograms
            for q in range(2):
                hsb = sbp.tile([P, 32 * NBINS], F32, tag="hsb", name="hsb",
                               bufs=4)
                nc.vector.tensor_copy(hsb, hists[q])
                nc.sync.dma_start(out_d.ap()[q], hsb)

    nc.compile()
    return nc


# ------------------------------------------------------------------- runner

_CACHE = {}


def _prepare(logits, labels, temperature, n_cores=N_CORES):
    labels = np.asarray(labels)
    key = hashlib.sha1(labels.tobytes()).hexdigest()
    if key in _CACHE:
        nc, core_idx = _CACHE[key]
    else:
        core_idx = build_plan(labels, n_cores)
        nc = build_program(N_TILES, n_cores)
        _CACHE[key] = (nc, core_idx)

    logits = np.asarray(logits, dtype=np.float32)
    tempr = np.broadcast_to(
        np.asarray(temperature, np.float32).ravel()[0:1], (P, 1)).copy()
    eye8 = np.concatenate([np.eye(128, dtype=F8NP)] * 2, axis=1)
    in_maps = []
    for c in range(n_cores):
        in_maps.append({
            "tempr": tempr,
            "eye8": eye8,
            "logits": build_core_slab(logits, labels, core_idx[c]),
        })
    return nc, in_maps


def finalize_host(hists, n_total=N_TOTAL):
    """hists: list of per-core [4, P, 32*NBINS] f32. Returns ECE f32 [1]."""
    j = np.arange(32)
    sc_cum = np.zeros(NBINS, np.float64)
    sa_cum = np.zeros(NBINS, np.float64)
    for h in hists:
        h5 = np.asarray(h, np.float64).reshape(4, P, NBINS, 32)
        for q in range(4):
            r0 = 32 * (q % 2)
            sc_cum += h5[q, r0 + j, :, j].sum(axis=0)
            sa_cum += h5[q, 64 + r0 + j, :, j].sum(axis=0)
    sc = sc_cum - np.concatenate([sc_cum[1:], [0.0]])
    sa = sa_cum - np.concatenate([sa_cum[1:], [0.0]])
    ece = np.abs(sc - sa).sum() / float(n_total)
    return np.asarray([ece], dtype=np.float32)


def _ensure_ntff_hook():
    """This container's antenv lacks axon_hooks; synthesize it and register
    the ctypes NTFF hook so trace=True works under axon."""
    try:
        import antenv.axon_hooks  # noqa: F401
        return
    except ImportError:
        pass
    import types

    import antenv

    mod = types.ModuleType("antenv.axon_hooks")
    _hook = [None]
    mod.set_axon_ntff_profile_hook = lambda h: _hook.__setitem__(0, h)
    mod.get_axon_ntff_profile_hook = lambda: _hook[0]
    sys.modules["antenv.axon_hooks"] = mod
    antenv.axon_hooks = mod
    try:
        from trn_agent_boot.trn_boot import _ntff_profile_via_ctypes
        mod.set_axon_ntff_profile_hook(
            _ntff_profile_via_ctypes("/opt/axon/libaxon_pjrt.so"))
    except Exception:
        pass


def run(logits, labels, temperature, n_total=None, trace=False,
        n_cores=N_CORES):
    if trace:
        _ensure_ntff_hook()
    if n_total is None:
        n_total = int(np.asarray(labels).shape[0])
    nc, in_maps = _prepare(logits, labels, temperature, n_cores)
    res = bass_utils.run_bass_kernel_spmd(
        nc, in_maps, core_ids=list(range(n_cores)), trace=trace)
    out = finalize_host([r["out"] for r in res.results], n_total)
    return out, res


def kernel(logits, labels, temperature):
    out, _ = run(logits, labels, temperature)
    return out


# revision 25
# speedup vs baseline: 1.0170x; 1.0170x over previous
"""ECE loss kernel for Trainium2, data-parallel over 8 NeuronCores.

Host side shards + permutes samples (binning is permutation invariant) into
128-sample "slots", swaps each row's label class into column 0 (max/sum are
class-permutation invariant, so accuracy becomes one column compare), casts
logits to bf16, and stores each 16-slot block's classes split into two
50-class halves so one fp8 DoubleRow matmul pair-sums them.

Device per tile (128 partitions x 128 slots x 100 classes):
  ScalarE: one exp(logits/T) instruction per tile -> E in fp8e4m3, plus a
           tiny per-pair exp of the raw per-sample max.
  DVE:     pairwise max tree on the RAW bf16 logits (independent of exp),
           fast reciprocal, conf/acc, 15 bin-threshold masks.
  PE:      softmax denominators via fp8 DoubleRow identity matmuls that
           pair-sum the two class halves while accumulating into PSUM with a
           stride-0 output dim (ping-pong over 2 partials: distance-2 RMW),
           plus the per-(slot, bin) histogram matmuls into 4 PSUM banks.
Pad rows are col0=0, rest=-80: conf=1.0 and acc=1 exactly, cancelling in the
bin-15 differential.  Each core DMAs its [4,128,480] histogram out; the host
extracts diagonal slot blocks and finishes the ECE reduction in float64.
"""

import hashlib
import sys

import numpy as np

sys.path.insert(0, "/opt/trn_rl_repo")

import ml_dtypes  # noqa: E402

from concourse import bacc, bass, mybir, tile  # noqa: E402
from concourse import bass_utils  # noqa: E402

P = 128          # partitions
G = 128          # slots (groups) per tile
TILE = P * G     # samples per tile (16384)
C = 100          # classes
H = C // 2       # class half (50)
BS = 16          # slots per block (DMA/exp/L1 pipeline granularity)
NBLK = G // BS   # blocks per tile
NBINS = 15
N_CORES = 8
BIG = 80.0       # pad logit; exp(-80) flushes to 0 in fp8/bf16 sums
N_TOTAL = 2_000_000
MM_SLOTS = 10    # slots per S-matmul (out iterations 10*50 <= 512)
G_LAST = 48      # short final tile (slots); 15*G + G_LAST = 1968 slots/core
NBLK_LAST = G_LAST // BS
N_TILES = 16
SLOTS_CORE = 15 * G + G_LAST

F32 = mybir.dt.float32
BF16 = mybir.dt.bfloat16
FP8 = mybir.dt.float8e4
BFNP = ml_dtypes.bfloat16
F8NP = ml_dtypes.float8_e4m3fn
AX = mybir.AxisListType
ALU = mybir.AluOpType
ACTF = mybir.ActivationFunctionType
PM = mybir.MatmulPerfMode


# ---------------------------------------------------------------- host layout

def build_plan(labels: np.ndarray, n_cores: int = N_CORES):
    """Deal samples round-robin to cores; slots mix labels freely (the
    label-column swap is per-row).  Returns per-core padded index arrays."""
    labels = np.asarray(labels).astype(np.int64).ravel()
    N = labels.shape[0]
    cap = SLOTS_CORE * P
    core_idx = []
    for c in range(n_cores):
        idx = np.arange(c, N, n_cores, dtype=np.int64)
        assert len(idx) <= cap
        buf = np.full(cap, -1, dtype=np.int64)
        buf[:len(idx)] = idx
        core_idx.append(buf)
    return core_idx


def build_core_slab(logits: np.ndarray, labels: np.ndarray,
                    idx: np.ndarray) -> np.ndarray:
    """One core's bf16 slab in device DMA order: 15 full tiles then one
    G_LAST tile, each [P, nblk, 2, BS, H], label class swapped into col 0."""
    cap = len(idx)
    arr = np.empty((cap, C), dtype=BFNP)
    arr[:, :] = logits[np.maximum(idx, 0)].astype(BFNP)
    ks = np.asarray(labels).astype(np.int64)[np.maximum(idx, 0)]
    rows = np.arange(cap)
    col0 = arr[rows, 0].copy()
    arr[rows, 0] = arr[rows, ks]
    arr[rows, ks] = col0
    pad_pos = np.nonzero(idx < 0)[0]
    if len(pad_pos):
        # col0=0, rest=-BIG: conf=1.0 and acc=1 exactly (cancels in bin 15)
        arr[pad_pos, :C] = BFNP(-BIG)
        arr[pad_pos, 0] = BFNP(0.0)
    full = arr[:15 * G * P].reshape(15, NBLK, BS, P, 2, H)
    full = np.ascontiguousarray(full.transpose(0, 3, 1, 4, 2, 5))
    last = arr[15 * G * P:].reshape(1, NBLK_LAST, BS, P, 2, H)
    last = np.ascontiguousarray(last.transpose(0, 3, 1, 4, 2, 5))
    return np.concatenate([full.reshape(-1, H), last.reshape(-1, H)])


# ------------------------------------------------------------- device program

def build_program(T: int, n_cores: int = N_CORES):
    nc = bacc.Bacc("TRN2", target_bir_lowering=False, debug=False,
                   num_devices=n_cores)

    logits_d = nc.dram_tensor("logits", [SLOTS_CORE * P * 2, H], BF16,
                              kind="ExternalInput")
    tempr_d = nc.dram_tensor("tempr", [P, 1], F32, kind="ExternalInput")
    eye8_d = nc.dram_tensor("eye8", [P, 2 * 128], FP8, kind="ExternalInput")
    out_d = nc.dram_tensor("out", [4, P, 32 * NBINS], F32,
                           kind="ExternalOutput")

    thr_imm = [float(np.float32(BFNP(b / NBINS))) for b in range(NBINS)]
    assert T % 2 == 0
    TF = G * C          # free elems per tile (12800)
    BF = BS * C         # free elems per block (1600)
    tile_G = [G] * (T - 1) + [G_LAST]
    tile_nblk = [NBLK] * (T - 1) + [NBLK_LAST]
    row_off = [0]
    for t in range(T):
        row_off.append(row_off[-1] + P * tile_nblk[t] * 2 * BS)

    with tile.TileContext(nc) as tc:
        with (
            tc.tile_pool(name="const", bufs=1) as const,
            tc.tile_pool(name="rawp", bufs=3) as rawp,
            tc.tile_pool(name="sb", bufs=2) as sbp,
            tc.tile_pool(name="ps", bufs=1, space="PSUM") as psp,
        ):
            zw = const.tile([P, 1], F32)
            nc.vector.memset(zw, 0.0)
            warm = const.tile([P, 1], F32)
            nc.scalar.activation(warm, zw, ACTF.Exp)
            tempr_t = const.tile([P, 1], F32)
            nc.sync.dma_start(tempr_t, tempr_d.ap())
            invT = const.tile([P, 1], F32)
            nc.vector.reciprocal(invT, tempr_t)
            eye8 = const.tile([P, 2 * 128], FP8)
            nc.sync.dma_start(eye8, eye8_d.ap())
            lhsT8 = eye8.rearrange("p (kt m) -> p kt m", kt=2)
            zeros_w = const.tile([P, 128], BF16)
            nc.vector.memset(zeros_w, 0.0)
            zdummy = const.tile([P, 512], BF16)
            nc.vector.memset(zdummy, 0.0)

            hists = [psp.tile([P, 32 * NBINS], F32, name=f"hist{q}")
                     for q in range(4)]  # [mainA, mainB, spareA, spareB]

            logits_ap = logits_d.ap()

            def tail_h(tp, h, Gt, Sp4, pack, emax_h):
                t = 2 * tp + h
                Sc = sbp.tile([P, G], F32, tag="Sch", name="Sch", bufs=2)
                nc.vector.tensor_copy(Sc[:, 0:Gt], Sp4[:, h, 1, 0:Gt])
                S2 = sbp.tile([P, G], F32, tag="S2h", name="S2h", bufs=2)
                nc.vector.tensor_tensor(S2[:, 0:Gt], Sp4[:, h, 0, 0:Gt],
                                        Sc[:, 0:Gt], op=ALU.add)
                R2 = sbp.tile([P, G], F32, tag="R2h", name="R2h", bufs=2)
                nc.vector.reciprocal_approx_fast(R2[:, 0:Gt], S2[:, 0:Gt])
                Rb2 = sbp.tile([P, G], BF16, tag="Rb2h", name="Rb2h", bufs=2)
                nc.vector.tensor_copy(Rb2[:, 0:Gt], R2[:, 0:Gt])
                conf = sbp.tile([P, G], BF16, tag="confh", name="confh",
                                bufs=2)
                nc.vector.tensor_tensor(conf[:, 0:Gt], emax_h[:, 0:Gt],
                                        Rb2[:, 0:Gt], op=ALU.mult)
                maskh = sbp.tile([P, NBINS * G], BF16, tag="maskh",
                                 name="maskh", bufs=2)
                for b in range(NBINS):
                    nc.vector.tensor_scalar(
                        maskh[:, b * G:b * G + Gt], conf[:, 0:Gt],
                        thr_imm[b], None, op0=ALU.is_gt)
                mh3 = maskh.rearrange("p (b g) -> p b g", b=NBINS)
                pack4 = pack.rearrange("p (r g) -> p r g", r=4)
                if Gt == G:
                    nc.vector.tensor_copy(
                        pack4[:, 0:3:2, :],
                        conf.rearrange("p (u g) -> p u g", u=2))
                else:
                    nc.vector.tensor_copy(pack[:, 0:Gt], conf[:, 0:Gt])
                for q in range(4):
                    if Gt < G and q >= 2:
                        continue
                    u = q // 2
                    w = min(32, max(0, Gt - 32 * q))
                    lhsT = pack[:, 128 * u:128 * u + 128]
                    rhs = mh3[:, :, 32 * q:32 * q + w]
                    ob = (hists[q] if w == 32 else
                          hists[q].rearrange("p (b j) -> p b j",
                                             b=NBINS)[:, :, 0:w])
                    stop_t = T - 2 if q >= 2 else T - 1
                    nc.tensor.matmul(ob, lhsT=lhsT, rhs=rhs,
                                     start=(t == 0), stop=(t == stop_t))
                if t == T - 2:
                    # banks 2,3 are complete: drain them now
                    for q in (2, 3):
                        hsb = sbp.tile([P, 32 * NBINS], F32, tag="hsb",
                                       name="hsb", bufs=4)
                        nc.vector.tensor_copy(hsb, hists[q])
                        nc.sync.dma_start(out_d.ap()[q], hsb)

            def tail(ctx):
                tp, m2, Sp4, pack_h, emax2 = ctx
                Sc = sbp.tile([P, 2 * G], F32, tag="Sc", name="Sc", bufs=2)
                Sc3 = Sc.rearrange("p (h g) -> p h g", h=2)
                nc.vector.tensor_copy(Sc3, Sp4[:, :, 1, :])
                S2 = sbp.tile([P, 2 * G], F32, tag="S2", name="S2", bufs=2)
                S23 = S2.rearrange("p (h g) -> p h g", h=2)
                nc.vector.tensor_tensor(S23, Sp4[:, :, 0, :], Sc3,
                                        op=ALU.add)
                R2 = sbp.tile([P, 2 * G], F32, tag="R2", name="R2", bufs=2)
                nc.vector.reciprocal_approx_fast(R2, S2)
                Rb2 = sbp.tile([P, 2 * G], BF16, tag="Rb2", name="Rb2",
                               bufs=2)
                nc.vector.tensor_copy(Rb2, R2)
                conf2 = sbp.tile([P, 2 * G], BF16, tag="conf2", name="conf2",
                                 bufs=2)
                nc.vector.tensor_tensor(conf2, emax2, Rb2, op=ALU.mult)

                # cumulative bin masks: 15 tensor_scalar is_gt (4x mode)
                mask2 = sbp.tile([P, NBINS * 2 * G], BF16, tag="mask2",
                                 name="mask2")
                for b in range(NBINS):
                    nc.vector.tensor_scalar(
                        mask2[:, b * 2 * G:(b + 1) * 2 * G], conf2,
                        thr_imm[b], None, op0=ALU.is_gt)
                m3 = mask2.rearrange("p (b g) -> p b g", b=NBINS)

                for h in range(2):
                    t = 2 * tp + h
                    Gt = tile_G[t]
                    pack = pack_h[h]
                    pack4 = pack.rearrange("p (r g) -> p r g", r=4)
                    if Gt == G:
                        nc.vector.tensor_copy(
                            pack4[:, 0:3:2, :],
                            conf2[:, h * G:(h + 1) * G].rearrange(
                                "p (u g) -> p u g", u=2))
                    else:
                        nc.vector.tensor_copy(pack[:, 0:Gt],
                                              conf2[:, h * G:h * G + Gt])

                    # histogram matmuls: slot-quarters into 4 PSUM banks,
                    # accumulated across all tiles.  Banks 2,3 get their last
                    # contribution from the T-2 tile (the short tile only
                    # fills banks 0,1).
                    for q in range(4):
                        if Gt < G and q >= 2:
                            continue
                        u = q // 2
                        w = min(32, max(0, Gt - 32 * q))
                        lhsT = pack[:, 128 * u:128 * u + 128]
                        rhs = m3[:, :, h * G + 32 * q:h * G + 32 * q + w]
                        ob = (hists[q] if w == 32 else
                              hists[q].rearrange("p (b j) -> p b j",
                                                 b=NBINS)[:, :, 0:w])
                        stop_t = T - 2 if q >= 2 else T - 1
                        nc.tensor.matmul(ob, lhsT=lhsT, rhs=rhs,
                                         start=(t == 0), stop=(t == stop_t))

            pending = None
            for tp in range(T // 2):
                m2 = sbp.tile([P, 2 * G], BF16, tag="m2", name="m2", bufs=2)
                # S partials for the pair: [h][two][g] f32, one PSUM bank
                Spair = psp.tile([P, 2 * 2 * G], F32, tag="Spair",
                                 name="Spair", bufs=2)
                # zero the bank on PE itself (keeps the S pipeline off DVE)
                nc.tensor.matmul(Spair, lhsT=zeros_w, rhs=zdummy,
                                 start=True, stop=False,
                                 skip_group_check=True)
                Sp4 = Spair.rearrange("p (h two g) -> p h two g", h=2, two=2)
                pack_h = []
                raw0_h = []
                emax_hh = []
                for h in range(2):
                    t = 2 * tp + h
                    Gt = tile_G[t]
                    nbt = tile_nblk[t]
                    raw = rawp.tile([P, Gt * C], BF16, tag="raw", name="raw",
                                    padded_shape=[P, TF])
                    raw5 = raw.rearrange("p (blk half s c) -> p blk half s c",
                                         blk=nbt, half=2, s=BS)
                    raw0_h.append(raw5[:, :, 0, :, 0])
                    E = sbp.tile([P, Gt * C], FP8, tag="E", name="E", bufs=3,
                                 padded_shape=[P, TF])
                    E3 = E.rearrange("p (blk half n) -> p blk half n",
                                     blk=nbt, half=2)
                    t1 = sbp.tile([P, Gt * H], BF16, tag="t1", name="t1",
                                  bufs=1, padded_shape=[P, G * H])
                    t13 = t1.rearrange("p (g c) -> p g c", g=Gt)
                    t14 = t1.rearrange("p (blk s c) -> p blk s c",
                                       blk=nbt, s=BS)

                    # Early tiles are sub-chunked so DVE/PE start earlier.
                    src = logits_ap[row_off[t]:row_off[t + 1],
                                    :].rearrange("(p f) c -> p (f c)", p=P)
                    nblk_per = {0: 2, 1: 4, 2: 4, T - 1: 1}.get(t, nbt)
                    for b0 in range(0, nbt, nblk_per):
                        b1 = b0 + nblk_per
                        fsl = slice(b0 * BF, b1 * BF)
                        nc.sync.dma_start(raw[:, fsl], src[:, fsl])
                        nc.scalar.activation(E[:, fsl], raw[:, fsl], ACTF.Exp,
                                             scale=invT)
                        # max tree L1 on raw halves (bf16 2x TT)
                        nc.vector.tensor_tensor(
                            t14[:, b0:b1], raw5[:, b0:b1, 0],
                            raw5[:, b0:b1, 1], op=ALU.max)

                    # ---- per-sample sum on PE: fp8 DoubleRow pair-summing
                    # matmuls, ping-pong partials (distance-2 RMW)
                    Sh = Sp4[:, h].transpose([0, 2, 1])  # [p, g, two]
                    for blk in range(nbt):
                        for s0 in range(0, BS, MM_SLOTS):
                            s1 = min(s0 + MM_SLOTS, BS)
                            g0 = blk * BS + s0
                            g1 = blk * BS + s1
                            rhs = E3[:, blk, :, s0 * H:s1 * H]
                            dst = (Sh[:, g0:g1, :].unsqueeze(2)
                                   .broadcast_to((P, g1 - g0, H // 2, 2)))
                            nc.tensor.matmul(dst, lhsT=lhsT8, rhs=rhs,
                                             start=False, stop=True,
                                             perf_mode=PM.DoubleRow,
                                             skip_group_check=True)

                    # ---- max tree L2..L7 (overlapped splits keep alignment)
                    t2 = sbp.tile([P, Gt * 26], BF16, tag="t2", name="t2",
                                  bufs=1, padded_shape=[P, G * 26])
                    t23 = t2.rearrange("p (g c) -> p g c", g=Gt)
                    nc.vector.tensor_tensor(t23, t13[:, :, 0:26],
                                            t13[:, :, 24:50], op=ALU.max)
                    t3 = sbp.tile([P, Gt * 14], BF16, tag="t3", name="t3",
                                  bufs=1, padded_shape=[P, G * 14])
                    t33 = t3.rearrange("p (g c) -> p g c", g=Gt)
                    nc.vector.tensor_tensor(t33, t23[:, :, 0:14],
                                            t23[:, :, 12:26], op=ALU.max)
                    t4 = sbp.tile([P, Gt * 8], BF16, tag="t4", name="t4",
                                  bufs=1, padded_shape=[P, G * 8])
                    t43 = t4.rearrange("p (g c) -> p g c", g=Gt)
                    nc.vector.tensor_tensor(t43, t33[:, :, 0:8],
                                            t33[:, :, 6:14], op=ALU.max)
                    t5 = sbp.tile([P, Gt * 4], BF16, tag="t5", name="t5",
                                  bufs=1, padded_shape=[P, G * 4])
                    t53 = t5.rearrange("p (g c) -> p g c", g=Gt)
                    nc.vector.tensor_tensor(t53, t43[:, :, 0:4],
                                            t43[:, :, 4:8], op=ALU.max)
                    t6 = sbp.tile([P, Gt * 2], BF16, tag="t6", name="t6",
                                  bufs=1, padded_shape=[P, G * 2])
                    t63 = t6.rearrange("p (g c) -> p g c", g=Gt)
                    nc.vector.tensor_tensor(t63, t53[:, :, 0:2],
                                            t53[:, :, 2:4], op=ALU.max)
                    nc.vector.tensor_tensor(m2[:, h * G:h * G + Gt],
                                            t63[:, :, 0:1].opt(),
                                            t63[:, :, 1:2].opt(), op=ALU.max)

                    # acc = raw[label] >= max(raw) (label class is column 0)
                    pack = sbp.tile([P, 2 * G], BF16, tag="pack", name="pack",
                                    bufs=4)
                    pack_h.append(pack)
                    pack4 = pack.rearrange("p (r g) -> p r g", r=4)
                    if Gt == G:
                        nc.vector.tensor_tensor(
                            pack4[:, 1:4:2, :].rearrange(
                                "p r (blk s) -> p r blk s", blk=NBLK // 2),
                            raw0_h[h].rearrange("p (u blk) s -> p u blk s",
                                                u=2),
                            m2[:, h * G:(h + 1) * G].rearrange(
                                "p (u blk s) -> p u blk s", u=2,
                                blk=NBLK // 2),
                            op=ALU.is_ge)
                        if tp == T // 2 - 1:
                            emax_h = sbp.tile([P, G], BF16, tag="emaxh",
                                              name="emax_h", bufs=2)
                            nc.scalar.activation(
                                emax_h[:, 0:Gt], m2[:, h * G:h * G + Gt],
                                ACTF.Exp, scale=invT)
                            emax_hh.append(emax_h)
                    else:
                        # short tile: all slots live in the u=0 half; zero
                        # the pack so dead lhsT columns contribute nothing
                        nc.vector.memset(pack, 0.0)
                        nc.vector.tensor_tensor(
                            pack[:, 64:64 + Gt].rearrange(
                                "p (blk s) -> p blk s", blk=nbt),
                            raw0_h[h],
                            m2[:, h * G:h * G + Gt].rearrange(
                                "p (blk s) -> p blk s", blk=nbt),
                            op=ALU.is_ge)
                        if tp == T // 2 - 1:
                            emax_h = sbp.tile([P, G], BF16, tag="emaxh",
                                              name="emax_h", bufs=2)
                            nc.scalar.activation(
                                emax_h[:, 0:Gt], m2[:, h * G:h * G + Gt],
                                ACTF.Exp, scale=invT)
                            emax_hh.append(emax_h)

                if tp < T // 2 - 1:
                    emax2 = sbp.tile([P, 2 * G], BF16, tag="emax2",
                                     name="emax2", bufs=2)
                    nc.scalar.activation(emax2, m2, ACTF.Exp, scale=invT)
                    if pending is not None:
                        tail(pending)
                    pending = (tp, m2, Sp4, pack_h, emax2)
                else:
                    # last pair: per-h emax already emitted in the h-loop
                    if pending is not None:
                        tail(pending)
                    for h in range(2):
                        tail_h(tp, h, tile_G[2 * tp + h], Sp4, pack_h[h],
                               emax_hh[h])

            # ---- finalize: drain the remaining histograms
            for q in range(2):
                hsb = sbp.tile([P, 32 * NBINS], F32, tag="hsb", name="hsb",
                               bufs=4)
                nc.vector.tensor_copy(hsb, hists[q])
                nc.sync.dma_start(out_d.ap()[q], hsb)

    nc.compile()
    return nc


# ------------------------------------------------------------------- runner

_CACHE = {}


def _prepare(logits, labels, temperature, n_cores=N_CORES):
    labels = np.asarray(labels)
    key = hashlib.sha1(labels.tobytes()).hexdigest()
    if key in _CACHE:
        nc, core_idx = _CACHE[key]
    else:
        core_idx = build_plan(labels, n_cores)
        nc = build_program(N_TILES, n_cores)
        _CACHE[key] = (nc, core_idx)

    logits = np.asarray(logits, dtype=np.float32)
    tempr = np.broadcast_to(
        np.asarray(temperature, np.float32).ravel()[0:1], (P, 1)).copy()
    eye8 = np.concatenate([np.eye(128, dtype=F8NP)] * 2, axis=1)
    in_maps = []
    for c in range(n_cores):
        in_maps.append({
            "tempr": tempr,
            "eye8": eye8,
            "logits": build_core_slab(logits, labels, core_idx[c]),
        })
    return nc, in_maps


def finalize_host(hists, n_total=N_TOTAL):
    """hists: list of per-core [4, P, 32*NBINS] f32. Returns ECE f32 [1]."""
    j = np.arange(32)
    sc_cum = np.zeros(NBINS, np.float64)
    sa_cum = np.zeros(NBINS, np.float64)
    for h in hists:
        h5 = np.asarray(h, np.float64).reshape(4, P, NBINS, 32)
        for q in range(4):
            r0 = 32 * (q % 2)
            sc_cum += h5[q, r0 + j, :, j].sum(axis=0)
            sa_cum += h5[q, 64 + r0 + j, :, j].sum(axis=0)
    sc = sc_cum - np.concatenate([sc_cum[1:], [0.0]])
    sa = sa_cum - np.concatenate([sa_cum[1:], [0.0]])
    ece = np.abs(sc - sa).sum() / float(n_total)
    return np.asarray([ece], dtype=np.float32)


def _ensure_ntff_hook():
    """This container's antenv lacks axon_hooks; synthesize it and register
    the ctypes NTFF hook so trace=True works under axon."""
    try:
        import antenv.axon_hooks  # noqa: F401
        return
    except ImportError:
        pass
    import types

    import antenv

    mod = types.ModuleType("antenv.axon_hooks")
    _hook = [None]
    mod.set_axon_ntff_profile_hook = lambda h: _hook.__setitem__(0, h)
    mod.get_axon_ntff_profile_hook = lambda: _hook[0]
    sys.modules["antenv.axon_hooks"] = mod
    antenv.axon_hooks = mod
    try:
        from trn_agent_boot.trn_boot import _ntff_profile_via_ctypes
        mod.set_axon_ntff_profile_hook(
            _ntff_profile_via_ctypes("/opt/axon/libaxon_pjrt.so"))
    except Exception:
        pass


def run(logits, labels, temperature, n_total=None, trace=False,
        n_cores=N_CORES):
    if trace:
        _ensure_ntff_hook()
    if n_total is None:
        n_total = int(np.asarray(labels).shape[0])
    nc, in_maps = _prepare(logits, labels, temperature, n_cores)
    res = bass_utils.run_bass_kernel_spmd(
        nc, in_maps, core_ids=list(range(n_cores)), trace=trace)
    out = finalize_host([r["out"] for r in res.results], n_total)
    return out, res


def kernel(logits, labels, temperature):
    out, _ = run(logits, labels, temperature)
    return out
